# revision 1
# baseline (speedup 1.0000x reference)
"""BatchSiren Trainium2 kernel.

B=2048 independent SIREN MLPs (2->32->32->3, sin activations, w0=30),
each evaluated on the same N=1024 coordinate grid.

Strategy (pure data parallel over 8 cores, 256 nets/core):
- 16 supergroups of 16 nets per core. Nets packed 16-at-a-time onto the
  128x128 PE array via 32x32 tile_position (independent concurrent tiles).
- All activations stay in [feature-partition, points-free] layout.
- sin(w0*z): weights pre-scaled by w0/2pi on host so matmuls produce the
  argument in CYCLE units; range reduction to one period via the
  magic-number round trick (ACT Identity+MAGIC / DVE tensor_scalar), then
  ACT Sin with scale=-2pi maps back to radians. The Sin table only covers
  [-pi, pi].
- Layer-3 output is produced point-major directly ([points, 4nets x 3ch])
  via col-tiled matmuls with block-diagonal w3, so the final DMA writes
  2KB-contiguous runs per partition.
"""
import numpy as np

import concourse.bacc as bacc
import concourse.bass as bass
import concourse.mybir as mybir
import concourse.tile as tile
from concourse import bass_utils

f32 = mybir.dt.float32
AF = mybir.ActivationFunctionType
ALU = mybir.AluOpType

W0 = 30.0
MAGIC = float(1.5 * 2 ** 23)
TWO_PI = float(2.0 * np.pi)
N_CORES = 8
B, N, IN, H, OUT = 2048, 1024, 2, 32, 3
BPC = B // N_CORES        # 256 batches per core
SGS = BPC // 16           # 16 supergroups of 16 nets
NH = N // 2               # 512 points per half

_compiled = None


def _build_module():
    nc = bacc.Bacc("TRN2", target_bir_lowering=False, debug=False)

    d_w1 = nc.dram_tensor("w1aug", [4, 3, 128 * SGS], f32, kind="ExternalInput")
    d_w2 = nc.dram_tensor("w2s", [4, 32, 128 * SGS], f32, kind="ExternalInput")
    d_w3 = nc.dram_tensor("w3blk", [4, 32, 48 * SGS], f32, kind="ExternalInput")
    d_sm = nc.dram_tensor("smalls", [128, 9 * SGS], f32, kind="ExternalInput")
    d_c = nc.dram_tensor("coords", [4, 3, N], f32, kind="ExternalInput")
    d_out = nc.dram_tensor("out", [SGS, 2, 48, NH], f32, kind="ExternalOutput")

    with tile.TileContext(nc) as tc:
        with tc.tile_pool(name="const", bufs=1) as cp, \
             tc.tile_pool(name="acts", bufs=2) as ap, \
             tc.tile_pool(name="outp", bufs=3) as op_, \
             tc.tile_pool(name="psA", bufs=1, space="PSUM") as psA, \
             tc.tile_pool(name="psB", bufs=2, space="PSUM") as psB:

            # ---- persistent constants ----
            w1sb = cp.tile([128, 128 * SGS], f32, tag="w1")
            for a in range(4):
                nc.sync.dma_start(w1sb[32 * a:32 * a + 3, :], d_w1[a])
            w2sb = cp.tile([128, 128 * SGS], f32, tag="w2")
            for b in range(4):
                nc.sync.dma_start(w2sb[32 * b:32 * b + 32, :], d_w2[b])
            w3sb = cp.tile([128, 48 * SGS], f32, tag="w3")
            for a in range(4):
                nc.sync.dma_start(w3sb[32 * a:32 * a + 32, :], d_w3[a])
            smalls = cp.tile([128, 9 * SGS], f32, tag="sm")
            nc.sync.dma_start(smalls[:], d_sm[:])
            c4 = cp.tile([128, N], f32, tag="c4")
            nc.vector.memset(c4[:], 0.0)  # rows 3-31 of each group MUST be 0
            for a in range(4):
                nc.sync.dma_start(c4[32 * a:32 * a + 3, :], d_c[a])
            magic = cp.tile([128, 1], f32, tag="mg")
            nc.vector.memset(magic[:], MAGIC)

            for sg in range(SGS):
                Q2 = ap.tile([128, 4096], f32, tag="Q2")
                H2 = ap.tile([128, 4096], f32, tag="H2")
                for h in range(2):
                    # ---- layer 1: 16 nets as 32x32 tiles, K=3 (w,b aug) ----
                    PZ1 = psA.tile([128, 2048], f32, tag="PZ1")
                    for a in range(4):
                        for b in range(4):
                            nc.tensor.matmul(
                                out=PZ1[32 * b:32 * b + 32, 512 * a:512 * a + 512],
                                lhsT=w1sb[32 * a:32 * a + 3,
                                          128 * sg + 32 * b:128 * sg + 32 * b + 32],
                                rhs=c4[32 * a:32 * a + 3, NH * h:NH * h + NH],
                                start=True, stop=True,
                                tile_position=(32 * a, 32 * b))
                    # round u1 to nearest integer (cycle count)
                    T1 = ap.tile([128, 2048], f32, tag="T1")
                    nc.scalar.activation(T1[:], PZ1[:], AF.Identity,
                                         bias=magic[:], scale=1.0)
                    Q1 = ap.tile([128, 2048], f32, tag="Q1")
                    nc.vector.scalar_tensor_tensor(
                        Q1[:], T1[:], MAGIC, PZ1[:], ALU.subtract, ALU.subtract)
                    H1 = ap.tile([128, 2048], f32, tag="H1")
                    nc.scalar.activation(H1[:], Q1[:], AF.Sin,
                                         bias=0.0, scale=-TWO_PI)

                    # ---- layer 2: 16 nets as 32x32 tiles, K=32 ----
                    PZ2a = psB.tile([128, 1024], f32, tag="B")
                    PZ2b = psB.tile([128, 1024], f32, tag="B")
                    for bt in range(4):
                        pt = PZ2a if bt < 2 else PZ2b
                        off = 512 * (bt % 2)
                        for a in range(4):
                            nc.tensor.matmul(
                                out=pt[32 * a:32 * a + 32, off:off + 512],
                                lhsT=w2sb[32 * bt:32 * bt + 32,
                                          128 * sg + 32 * a:128 * sg + 32 * a + 32],
                                rhs=H1[32 * bt:32 * bt + 32, 512 * a:512 * a + 512],
                                start=True, stop=True,
                                tile_position=(32 * bt, 32 * a))
                    # t2 = (z + b2cyc) + MAGIC, per segment (bias varies by bt)
                    T2 = ap.tile([128, 2048], f32, tag="T2")
                    for bt in range(4):
                        pt = PZ2a if bt < 2 else PZ2b
                        off = 512 * (bt % 2)
                        nc.vector.tensor_scalar(
                            T2[:, 512 * bt:512 * bt + 512],
                            pt[:, off:off + 512],
                            smalls[:, 9 * sg + bt:9 * sg + bt + 1], MAGIC,
                            ALU.add, ALU.add)
                    # q2 = (t2 - MAGIC) - z ; scatter into Q2 as (bt, h, n)
                    for half_ps, pt, bts in ((0, PZ2a, (0, 1)), (1, PZ2b, (2, 3))):
                        t2v = T2[:].rearrange("p (b n) -> p b n", b=4)[
                            :, bts[0]:bts[1] + 1, :]
                        q2v = Q2[:].rearrange("p (b g n) -> p b g n", b=4, g=2)[
                            :, bts[0]:bts[1] + 1, h, :]
                        z2v = pt[:].rearrange("p (b n) -> p b n", b=2)
                        nc.vector.scalar_tensor_tensor(
                            q2v, t2v, MAGIC, z2v, ALU.subtract, ALU.subtract)

                # ---- sin2, batched over both halves per bt (shared bias) ----
                for bt in range(4):
                    nc.scalar.activation(
                        H2[:, 1024 * bt:1024 * bt + 1024],
                        Q2[:, 1024 * bt:1024 * bt + 1024],
                        AF.Sin,
                        bias=smalls[:, 9 * sg + 4 + bt:9 * sg + 4 + bt + 1],
                        scale=-TWO_PI)

                # ---- layer 3: col-tiled, block-diag w3 -> point-major out ----
                for h in range(2):
                    PC = psB.tile([128, 512], f32, tag="B")
                    for bt in range(4):
                        nc.tensor.matmul(
                            out=PC[32 * bt:32 * bt + 12, :],
                            lhsT=w3sb[:, 48 * sg + 12 * bt:48 * sg + 12 * bt + 12],
                            rhs=H2[:, 1024 * bt + 512 * h:1024 * bt + 512 * h + 512],
                            start=True, stop=True,
                            tile_position=(0, 32 * bt))
                    OT = op_.tile([128, 512], f32, tag="OT")
                    nc.scalar.activation(OT[:], PC[:], AF.Identity,
                                         bias=smalls[:, 9 * sg + 8:9 * sg + 9],
                                         scale=1.0)
                    for bt in range(4):
                        nc.sync.dma_start(
                            d_out[sg, h, 12 * bt:12 * bt + 12, :],
                            OT[32 * bt:32 * bt + 12, :])

    nc.compile()
    return nc


def _prep_core_inputs(w1, b1, w2, b2, w3, b3, coords, core):
    s = np.float32(W0 / TWO_PI)
    B0 = core * BPC
    sl = slice(B0, B0 + BPC)

    # [sg, a, b] batch grid
    w1c = w1[sl, :, :, 0].reshape(SGS, 4, 4, IN, H)
    b1c = b1[sl, :, 0].reshape(SGS, 4, 4, H)
    aug = np.concatenate([w1c, b1c[:, :, :, None, :]], axis=3) * s  # [sg,a,b,3,32]
    w1aug = np.ascontiguousarray(
        aug.transpose(1, 3, 0, 2, 4).reshape(4, 3, SGS * 128)).astype(np.float32)

    w2c = (w2[sl, :, :, 0] * s).reshape(SGS, 4, 4, H, H)  # [sg,a,b,i,o]
    w2s = np.ascontiguousarray(
        w2c.transpose(2, 3, 0, 1, 4).reshape(4, 32, SGS * 128)).astype(np.float32)

    w3c = w3[sl, :, :, 0].reshape(SGS, 4, 4, H, OUT)  # [sg,a,b,i,c]
    blk = np.zeros((SGS, 4, 4, H, 4, OUT), np.float32)  # [sg,a,b,i,a',c]
    for a in range(4):
        blk[:, a, :, :, a, :] = w3c[:, a]
    # free index inside a sg block: 12*b + 3*a' + c  -> order [b, a', c]
    w3blk = np.ascontiguousarray(
        blk.transpose(1, 3, 0, 2, 4, 5).reshape(4, 32, SGS * 48)).astype(np.float32)

    b2c = b2[sl, :, 0].reshape(SGS, 4, 4, H)  # [sg,a,b,o]
    b3c = b3[sl, :, 0].reshape(SGS, 4, 4, OUT)  # [sg,a,b,c]
    smalls = np.zeros((128, SGS, 9), np.float32)
    p = np.arange(128)
    a_idx, o_idx = p // 32, p % 32
    for bt in range(4):
        # b2 in cycles at col bt ; radians at col 4+bt; partition 32a+o
        smalls[:, :, bt] = (b2c[:, a_idx, bt, o_idx] * s).T
        smalls[:, :, 4 + bt] = (b2c[:, a_idx, bt, o_idx] * np.float32(W0)).T
    # b3: partition 32*bt + 3*a + c
    bt_idx, m_idx = p // 32, p % 32
    valid = m_idx < 12
    a3, c3 = m_idx // 3, m_idx % 3
    for pi in range(128):
        if valid[pi]:
            smalls[pi, :, 8] = b3c[:, a3[pi], bt_idx[pi], c3[pi]]
    smalls = np.ascontiguousarray(smalls.reshape(128, SGS * 9))

    ch = np.zeros((4, 3, N), np.float32)
    ch[:, :IN, :] = coords.T[None, :, :]
    ch[:, IN, :] = 1.0

    return {"w1aug": w1aug, "w2s": w2s, "w3blk": w3blk,
            "smalls": smalls, "coords": ch}


def _unshard(res_list):
    outs = []
    for r in res_list:
        o = r["out"].reshape(SGS, 2, 4, 4, OUT, NH)      # [sg,h,bt,a,c,n]
        o = o.transpose(0, 3, 2, 1, 5, 4)                # [sg,a,bt,h,n,c]
        outs.append(np.ascontiguousarray(o.reshape(BPC, N, OUT)))
    return np.concatenate(outs, axis=0)


def _run(inputs, trace=False, trace_kwargs=None):
    global _compiled
    if _compiled is None:
        _compiled = _build_module()
    nc = _compiled
    arrs = {k: np.asarray(v, dtype=np.float32) for k, v in inputs.items()}
    in_maps = [_prep_core_inputs(arrs["w1"], arrs["b1"], arrs["w2"], arrs["b2"],
                                 arrs["w3"], arrs["b3"], arrs["coords"], c)
               for c in range(N_CORES)]
    kw = {}
    if trace:
        kw["trace"] = True
        if trace_kwargs:
            kw.update(trace_kwargs)
    res = bass_utils.run_bass_kernel_spmd(nc, in_maps, core_ids=list(range(N_CORES)),
                                          **kw)
    out = _unshard(res.results)
    return out, res


def kernel(**inputs):
    out, _ = _run(inputs, trace=False)
    return out
